# revision 45
# baseline (speedup 1.0000x reference)
"""Multi-head attention kernel for 8 Trainium2 NeuronCores (v3).

Problem: B=2, S=2048, D=1024, H=16 heads (head_dim 64).
Sharding: data-parallel over batch (2) x tensor-parallel over heads (4 groups
of 4 heads). Core c handles batch c//4, heads [4*(c%4), 4*(c%4)+4).
Each core computes a partial [S, D] output (its heads' contribution through
Wo); the host sums the 4 TP partials per batch.

v3 redesign vs v2 (v2 ran projections serially before attention, leaving the
ACT engine idle for the first ~50us of a ~257us kernel):
- The exp stream IS the kernel: 128 ACT exps of [128,1024] score tiles
  (~142us serial ACT work). Everything else hides under it.
- Loops run head-pair-major ((h0,q0..q3) then (h1,q0..q3)) so only the ot0
  halves of the K/Q projections gate the first 4 loops; ot1 halves, V-hp1
  and all Wo work fill later loops.
- First exp fires ~9us in, gated only by K-chunk0-ot0 + Q-chunk0-ot0
  (DMAs split across the three DGE rings) + an 8-matmul projection each.
- PV matmuls are deferred through a V-readiness-gated FIFO: exps never wait
  for V. `at` probability tiles are retained (deep ring) until PV drains.
- Projections/Wo run as slot-gated filler closures in the PE stream, sized
  ~4 matmuls, borrowing the idle PV PSUM bank set. Fillers a loop's scores
  depend on are scheduled strictly before that loop's emission point
  (program order = dependency order in Tile).
- Softmax 1/den off the in-order ACT queue: DVE reciprocal on the PSUM den
  row, DRAM-roundtrip partition broadcast, DVE normalize-multiplies; the
  norm emission chains off the loop's 16th PV matmul. Only the tail norm
  stays on ACT (Ln/Exp + PE rank-1 broadcast) for latency.
- fp16 everywhere (fp32 PSUM). fp8/DoubleRow was measured (micro.py) but
  rejected: e4m3 quantization on the incoherently-summed PV/Wo paths adds
  ~2-5% relative sigma, over the 2e-2 absmax gate.
"""
import sys

sys.path.insert(0, "/opt/trn_rl_repo")

from collections import deque

import numpy as np

import concourse.bass as bass
import concourse.tile as tile
from concourse import mybir
from concourse import bass_utils

# no fish share in this container; only used when tracing
bass_utils.upload_artifacts = lambda tmpdir: f"local://{tmpdir}"

B, S, D, H = 2, 2048, 1024, 16
HD = 64          # head dim
HL = 4           # heads per core (local)
DL = HL * HD     # local projection dim = 256
N_CORES = 8
SC = 4           # s-chunks of 512 for projections
QC = 4           # q-chunks of 512 for attention
KT = 16          # k-tiles of 128

dt32 = mybir.dt.float32
dtb = mybir.dt.float16

TRACE = False           # set by test.py for profiling runs
LAST_EXEC_NS = None     # stashed by kernel() when TRACE


# ---------------------------------------------------------------- wait split
def _split_waits(nc):
    """Walrus codegen accepts at most one sync wait per instruction on this
    toolchain; move excess waits onto same-engine NoOps inserted before the
    overloaded instruction (engine program order makes this equivalent)."""
    n = 0
    for bb_wrap in nc.main_func.blocks:
        bb = bb_wrap if not hasattr(bb_wrap, "bb") else bb_wrap.bb
        insts = list(bb.instructions)
        out = []
        for ins in insts:
            si = ins.sync_info
            waits = list(si.on_wait) if si is not None else []
            if len(waits) > 1:
                for w in waits[:-1]:
                    nop = mybir.InstNoOp(
                        name=nc.get_next_instruction_name(), ins=[], outs=[]
                    )
                    nop.engine = ins.engine
                    nop.sync_info = mybir.SyncInfo(on_wait=[w], on_update=[])
                    nc.register_instruction(nop)
                    out.append(nop)
                    n += 1
                ins.sync_info = mybir.SyncInfo(
                    on_wait=waits[-1:], on_update=list(si.on_update)
                )
            out.append(ins)
        if len(out) != len(insts):
            bb.instructions = out
    return n


# ---------------------------------------------------------------- program
_PROGRAM = None


def _build_program():
    nc = bass.Bass()
    # All inputs/outputs are pre-tiled on the host so every DMA is a
    # fully contiguous block: strided DMAs cost 2-7us of SEQUENCER time to
    # issue (per-row descriptors) and transfer far below HBM rate.
    xq = nc.declare_dram_parameter("xq", [SC, 128, 2, 4, 512], dtb, isOutput=False)
    xk = nc.declare_dram_parameter("xk", [SC, 128, 2, 4, 512], dtb, isOutput=False)
    xv = nc.declare_dram_parameter("xv", [SC, 128, 2, 4, 512], dtb, isOutput=False)
    # wq+wk packed ot-major: one 0.5MB DMA per ot half
    wqk = nc.declare_dram_parameter("wqk", [2, 128, 2, 8, 128], dtb, isOutput=False)
    wv = nc.declare_dram_parameter("wv", [2, 128, 8, 128], dtb, isOutput=False)
    wo = nc.declare_dram_parameter("wo", [128, 2, D], dtb, isOutput=False)
    out = nc.declare_dram_parameter("out", [QC, 4, 2, 128, 512], dtb, isOutput=True)

    with tile.TileContext(nc) as tc:
        with tc.tile_pool(name="const", bufs=1) as const, \
             tc.tile_pool(name="persist", bufs=1) as persist, \
             tc.tile_pool(name="xin", bufs=1) as xin, \
             tc.tile_pool(name="attn", bufs=1) as attn, \
             tc.tile_pool(name="recp", bufs=2) as recp, \
             tc.tile_pool(name="acc", bufs=3) as acc, \
             tc.tile_pool(name="outsb", bufs=4) as outsb, \
             tc.tile_pool(name="dram", bufs=1, space="DRAM") as dram, \
             tc.tile_pool(name="psum", bufs=1, space="PSUM") as psum:

            # ---- resident weights & activations ----
            # [p, ot, which(0=K,1=Q), kc, o]
            wqks = const.tile([128, 2, 2, 8, 128], dtb, tag="wqk")
            wvs = const.tile([128, 2, 8, 128], dtb, tag="wv")  # [p, hp, kc, o]
            wos = const.tile([128, 2, D], dtb, tag="wo")
            qts = persist.tile([128, 2, S], dtb, tag="qts")  # [o%128, o//128, s]
            kts = persist.tile([128, 2, S], dtb, tag="kts")
            # V with a trailing ones column: stationary [v | 1] gives the
            # softmax denominator as PSUM row 64 of the PV accumulation.
            vtsE = persist.tile([128, KT, 2, 66], dtb, tag="vtsE")  # even heads
            vtsO = persist.tile([128, KT, 2, 66], dtb, tag="vtsO")  # odd heads
            # ctx2: head-pair ctx stacked across partitions (even head rows
            # 0-63 from the DVE normalize; odd head rows 64-127 filled by an
            # SBUF->SBUF partition-shift DMA from ctxO) so Wo runs as
            # 2x K=128 matmuls per out tile instead of 4x K=64.
            ctx2 = persist.tile([128, 8, 512], dtb, tag="ctx2")  # slot = qc*2+hp
            ctxO = persist.tile([HD, 2, 512], dtb, tag="ctxO")  # slot%2 ring

            # ---- input DMAs, split across the three DGE rings ----
            def x_chunk(xdram, c, tag, queue):
                t = xin.tile([128, 2, 4, 512], dtb, tag=tag, name=tag, bufs=1)
                queue.dma_start(out=t[:], in_=xdram[c])
                return t

            xkc, xqc, xvc = {}, {}, {}
            # sync ring
            xkc[0] = x_chunk(xk, 0, "xk0", nc.sync)
            xkc[2] = x_chunk(xk, 2, "xk2", nc.sync)
            xvc[0] = x_chunk(xv, 0, "xv0", nc.sync)
            xvc[2] = x_chunk(xv, 2, "xv2", nc.sync)
            xqc[2] = x_chunk(xq, 2, "xq2", nc.sync)
            nc.sync.dma_start(out=wos[:], in_=wo[:])
            # scalar (ACT) ring: weights ONLY, all issued before the exp
            # stream starts -- a blocked DMA issue on this ring stalls the
            # ACT sequencer and with it every exp.
            nc.scalar.dma_start(out=wqks[:, 0], in_=wqk[0])
            nc.scalar.dma_start(out=wqks[:, 1], in_=wqk[1])
            nc.scalar.dma_start(out=wvs[:, 0], in_=wv[0])
            nc.scalar.dma_start(out=wvs[:, 1], in_=wv[1])
            # gpsimd ring
            xqc[0] = x_chunk(xq, 0, "xq0", nc.gpsimd)
            xkc[1] = x_chunk(xk, 1, "xk1", nc.gpsimd)
            xqc[1] = x_chunk(xq, 1, "xq1", nc.gpsimd)
            xkc[3] = x_chunk(xk, 3, "xk3", nc.gpsimd)
            xvc[1] = x_chunk(xv, 1, "xv1", nc.gpsimd)
            xvc[3] = x_chunk(xv, 3, "xv3", nc.gpsimd)
            xqc[3] = x_chunk(xq, 3, "xq3", nc.gpsimd)

            # ---- constants ----
            nc.vector.memset(vtsE[:], 1.0)
            nc.vector.memset(vtsO[:], 1.0)
            # bias const for the tail's exp-based reciprocal: -12*ln(2)
            nbias = const.tile([128, 1], dt32, tag="nbias")
            nc.vector.memset(nbias[:], -12.0 * 0.6931471805599453)
            # f16 ones for the tail's PE-broadcast of 1/den
            ones16 = const.tile([128, HD], dtb, tag="ones16")
            nc.vector.memset(ones16[:], 1.0)
            # dummy exp+ln to pull the act-table load into the preamble
            warm = const.tile([128, 2], dt32, tag="warm")
            nc.vector.memset(warm[:], 1.0)
            nc.scalar.activation(out=warm[0:1, 0:1], in_=warm[0:1, 0:1],
                                 func=mybir.ActivationFunctionType.Exp)
            nc.scalar.activation(out=warm[0:1, 1:2], in_=warm[0:1, 1:2],
                                 func=mybir.ActivationFunctionType.Ln)

            # ---- filler machinery ----
            # g = global slot = loop*16 + kt. fillers: slot-gated closures
            # run in the PE stream. pvq: PV matmul closures gated on their V
            # chunk's projection having RUN (v_ready), FIFO order.
            fillers = []          # list of [min_g, ready_fn, fn]
            pvq = deque()         # (hp, chunk, fn)
            v_ready = {(hp, c): False for hp in range(2) for c in range(SC)}
            cur_g = [0]

            def drain_pv(budget):
                done = 0
                while pvq and done < budget:
                    hp, c, fn = pvq[0]
                    if not v_ready[(hp, c)]:
                        break
                    pvq.popleft()
                    fn()
                    done += 1
                return done

            def drain_fill(budget):
                # Pick the eligible filler with the SMALLEST min_g: norm-fin
                # closures must preempt bank-borrowing fillers pushed earlier.
                done = 0
                while done < budget:
                    best, best_i = None, -1
                    for i, (mg, rdy, fn) in enumerate(fillers):
                        if mg > cur_g[0]:
                            continue
                        if rdy is not None and not rdy():
                            continue
                        if best is None or mg < best:
                            best, best_i = mg, i
                    if best_i < 0:
                        break
                    mg, rdy, fn = fillers.pop(best_i)
                    fn()
                    done += 1
                return done

            def push(min_g, fn, ready=None):
                fillers.append([min_g, ready, fn])

            def drain_everything():
                cur_g[0] = 10 ** 9
                guard = 0
                while (pvq or fillers) and guard < 10000:
                    drain_pv(len(pvq) + 1)
                    drain_fill(len(fillers) + 1)
                    guard += 1

            # PSUM: sc0/sc1 are [128,1024] score double-buffers (4 banks).
            # pvA/pvB are THE PV accumulator pair for every loop: PV runs in
            # groups of 8 kt, each group evict/added into an SBUF fp16
            # accumulator (DVE), so the banks free deterministically inline.
            # pvA2/pvB2 are a dedicated filler pair, always borrowable.
            fill_rot = [0]

            def next_fill_tag():
                if cur_g[0] >= 8 * KT:  # tail: rotate all four
                    tags = ("pvA", "pvB", "pvA2", "pvB2")
                    t = tags[fill_rot[0] % 4]
                else:
                    t = ("pvA2", "pvB2")[fill_rot[0] % 2]
                fill_rot[0] += 1
                return t

            # ---- projection builders ----
            def proj_qk_half(wsel, xct, dst, c, ot, box, half):
                """wsel: 0=K, 1=Q. half 0: kc0-3 into a fresh borrowed
                bank; half 1: kc4-7 + cast eviction."""
                def go():
                    if "p" not in box:
                        box["p"] = psum.tile([128, 512], dt32,
                                             tag=next_fill_tag(),
                                             name=f"pj{c}o{ot}")
                    p = box["p"]
                    kcs = range(0, 4) if half == 0 else range(4, 8)
                    for kc in kcs:
                        nc.tensor.matmul(
                            p[:],
                            wqks[:, ot, wsel, kc, :],
                            xct[:, kc // 4, kc % 4, :],
                            start=(kc == 0),
                            stop=(kc == 7),
                            skip_group_check=True,
                        )
                    if half == 1:
                        nc.vector.tensor_copy(
                            dst[:, ot, c * 512:(c + 1) * 512], p[:])
                return go

            def push_proj_qk(wsel, chunk_map, dst, c, ot, g0, g1):
                box = {}

                def half(h):
                    def go():
                        proj_qk_half(wsel, chunk_map[c], dst, c, ot, box, h)()
                    return go

                push(g0, half(0))
                push(g1, half(1))

            def proj_v_half(c, hp, box, half):
                """V proj produces [s-sub, head dl] (s on partitions). The hp
                half moves wvs cols [hp*128:(hp+1)*128] (heads 2hp, 2hp+1).
                half 0: s-subtiles 0,1; half 1: subtiles 2,3 + v_ready."""
                xct = xvc[c]

                def go():
                    if "p" not in box:
                        box["p"] = psum.tile([128, 512], dt32,
                                             tag=next_fill_tag(),
                                             name=f"pv{c}h{hp}")
                    p = box["p"]
                    sts = (0, 1) if half == 0 else (2, 3)
                    for st in sts:
                        for kc in range(8):
                            nc.tensor.matmul(
                                p[:, st * 128:(st + 1) * 128],
                                xct[:, kc // 4, kc % 4, st * 128:(st + 1) * 128],
                                wvs[:, hp, kc, :],
                                start=(kc == 0),
                                stop=(kc == 7),
                                skip_group_check=True,
                            )
                        idx = c * 4 + st
                        ph = p[:, st * 128:(st + 1) * 128].rearrange(
                            "p (h d) -> p h d", h=2)
                        nc.vector.tensor_copy(vtsE[:, idx, hp, 0:HD], ph[:, 0, :])
                        nc.vector.tensor_copy(vtsO[:, idx, hp, 0:HD], ph[:, 1, :])
                    if half == 1:
                        v_ready[(hp, c)] = True
                return go

            def push_proj_v(c, hp, g0, g1):
                box = {}
                push(g0, proj_v_half(c, hp, box, 0))
                push(g1, proj_v_half(c, hp, box, 1))

            # ---- softmax normalization (off-ACT path) ----
            def norm_start_finish(li, qc, hp, cacc):
                """Emit DVE reciprocal + DRAM-roundtrip broadcast now (called
                right after the loop's last PV group lands in the SBUF
                accumulator); schedule the normalize-multiplies + partition
                shift + (for hp1) Wo as fillers a few slots later."""
                slot = qc * 2 + hp
                rec32 = recp.tile([128, 2, 512], dt32, tag="rec32",
                                  name=f"rec{slot}")
                # The DVE reciprocal is ~6.4ns/elem; emitting it whole
                # blocks the DVE for 6.6us and stalls PE fillers WAR-ing on
                # DVE evictions. Split into 2 chunks drained between
                # fillers; the broadcast roundtrip goes fp16.
                g = cur_g[0]
                for hh in range(2):

                    def rc(hh=hh):
                        nc.vector.reciprocal(
                            rec32[64:65, hh, :], cacc[64:65, hh, :])
                    push(g + 2 + 2 * hh, rc)

                rec16 = recp.tile([128, 2, 512], dtb, tag="rec16",
                                  name=f"rec16_{slot}")
                recd = dram.tile([2, 512], dtb, tag=f"recd{slot}",
                                 name=f"recd{slot}")
                denbS = recp.tile([HD, 2, 512], dtb, tag="denbS",
                                  name=f"denbS{slot}")

                def bcast():
                    nc.vector.tensor_copy(rec16[64:65, :, :],
                                          rec32[64:65, :, :])
                    nc.sync.dma_start(out=recd[:], in_=rec16[64:65, :, :])
                    for hh in range(2):
                        row = recd[hh:hh + 1, :]
                        bc = bass.AP(
                            tensor=row.tensor,
                            offset=row.offset,
                            ap=[[0, HD]] + [list(x) for x in row.ap[1:]],
                        )
                        nc.sync.dma_start(out=denbS[:, hh, :], in_=bc)
                push(g + 6, bcast)

                def fin():
                    nc.gpsimd.tensor_mul(
                        ctx2[0:HD, slot, :], cacc[0:HD, 0, :], denbS[:, 0, :])
                    nc.gpsimd.tensor_mul(
                        ctxO[:, slot % 2, :], cacc[0:HD, 1, :], denbS[:, 1, :])
                    nc.sync.dma_start(
                        out=ctx2[HD:128, slot, :], in_=ctxO[:, slot % 2, :])
                    if hp == 1:
                        push_wo(qc, cur_g[0] + 2)
                push(g + 8, fin)

            def do_norm_tail(qc, hp, cacc):
                # tail-only: ACT is idle after the last exp and PE is nearly
                # free; Ln/Exp reciprocal + rank-1 PE broadcast beats the
                # DVE-recip + DRAM-roundtrip latency chain.
                slot = qc * 2 + hp
                lnden = recp.tile([128, 2, 512], dt32, tag="rec32", name="lndent")
                rec16 = recp.tile([128, 2, 512], dtb, tag="rec16t", name="rec16t", bufs=1)
                nc.scalar.activation(
                    out=lnden[64:65, 0, :], in_=cacc[64:65, 0, :],
                    func=mybir.ActivationFunctionType.Ln, scale=2.0 ** -12,
                )
                nc.scalar.activation(
                    out=lnden[64:65, 1, :], in_=cacc[64:65, 1, :],
                    func=mybir.ActivationFunctionType.Ln, scale=2.0 ** -12,
                )
                nc.scalar.activation(
                    out=rec16[64:65, :, :], in_=lnden[64:65, :, :],
                    func=mybir.ActivationFunctionType.Exp, scale=-1.0,
                    bias=nbias[64:65, :],
                )
                denbE = psum.tile([128, 512], dt32, tag="pvA", name="denbE")
                denbO = psum.tile([128, 512], dt32, tag="pvB", name="denbO")
                nc.tensor.matmul(
                    denbE[0:HD, :], ones16[64:65, :], rec16[64:65, 0, :],
                    start=True, stop=True, skip_group_check=True,
                )
                nc.tensor.matmul(
                    denbO[0:HD, :], ones16[64:65, :], rec16[64:65, 1, :],
                    start=True, stop=True, skip_group_check=True,
                )
                denbS = recp.tile([HD, 2, 512], dt32, tag="denbSt", name="denbSt", bufs=1)
                nc.vector.tensor_copy(denbS[:, 0, :], denbE[0:HD, :])
                nc.vector.tensor_copy(denbS[:, 1, :], denbO[0:HD, :])
                nc.vector.tensor_mul(
                    ctx2[0:HD, slot, :], cacc[0:HD, 0, :], denbS[:, 0, :])
                nc.vector.tensor_mul(
                    ctxO[:, slot % 2, :], cacc[0:HD, 1, :], denbS[:, 1, :])
                # split the partition-shift over two rings for tail latency
                nc.sync.dma_start(
                    out=ctx2[HD:HD + 32, slot, :], in_=ctxO[0:32, slot % 2, :])
                nc.gpsimd.dma_start(
                    out=ctx2[HD + 32:128, slot, :], in_=ctxO[32:64, slot % 2, :])

            # ---- Wo fillers ----
            def push_wo(qc, g0, tail=False):
                for idx in range(8):
                    t, jc = idx // 2, idx % 2

                    def mk(t, jc, idx):
                        def go():
                            po = psum.tile(
                                [128, 512], dt32,
                                tag=next_fill_tag(), name=f"po{qc}")
                            for p in range(2):
                                nc.tensor.matmul(
                                    po[:],
                                    ctx2[:, qc * 2 + p, t * 128:(t + 1) * 128],
                                    wos[:, p, jc * 512:(jc + 1) * 512],
                                    start=(p == 0),
                                    stop=(p == 1),
                                    skip_group_check=True,
                                )
                            ob = outsb.tile([128, 512], dtb, tag="ob", name="ob")
                            if tail:
                                # ACT engine is idle after the last exp; use
                                # it for eviction so the tail isn't
                                # DVE-serialized, and split out-DMAs over
                                # two rings
                                nc.scalar.copy(ob[:], po[:])
                            else:
                                nc.vector.tensor_copy(ob[:], po[:])
                            (nc.sync if (tail and idx % 2) else nc.gpsimd).dma_start(
                                out=out[qc, t, jc], in_=ob[:],
                            )
                        return go

                    push(g0 + (3 * idx) // 2, mk(t, jc, idx))

            # ---- attention loop ----
            sc_tags = ("sc0", "sc1")

            def attn_loop(li, qc, hp, norm_cb):
                # sweep fillers scheduled before this loop's start so the
                # scores below are emitted after (= depend on) their
                # projections
                cur_g[0] = li * KT
                drain_fill(len(fillers) + 1)
                budget = 2 if li <= 1 or li == 7 else 1

                # PV accumulates in pvA/pvB in two groups of 8 kt; each
                # group is evicted (group 0: copy, group 1: in-place add)
                # into the SBUF fp16 accumulator, freeing the banks inline.
                cacc = acc.tile([65, 2, 512], dtb, tag="cacc",
                                name=f"cacc{li}")
                q0 = qc * 512
                nmm = [0]
                pvt = {}

                def mk_pv(kt, at):
                    def go():
                        w = nmm[0]
                        nmm[0] += 1
                        if w % 8 == 0:
                            pvt["E"] = psum.tile([128, 512], dt32, tag="pvA",
                                                 name=f"pvE{li}g{w // 8}")
                            pvt["O"] = psum.tile([128, 512], dt32, tag="pvB",
                                                 name=f"pvO{li}g{w // 8}")
                        first, last = w % 8 == 0, w % 8 == 7
                        nc.tensor.matmul(
                            pvt["E"][0:65, :], vtsE[:, kt, hp, 0:65],
                            at[:, 0:512],
                            start=first, stop=last, skip_group_check=True,
                        )
                        nc.tensor.matmul(
                            pvt["O"][0:65, :], vtsO[:, kt, hp, 0:65],
                            at[:, 512:1024],
                            start=first, stop=last, skip_group_check=True,
                        )
                        if w % 8 == 7:
                            if w // 8 == 0:
                                nc.vector.tensor_copy(
                                    cacc[:, 0, :], pvt["E"][0:65, :])
                                nc.vector.tensor_copy(
                                    cacc[:, 1, :], pvt["O"][0:65, :])
                            else:
                                nc.vector.tensor_add(
                                    cacc[:, 0, :], pvt["E"][0:65, :],
                                    cacc[:, 0, :])
                                nc.vector.tensor_add(
                                    cacc[:, 1, :], pvt["O"][0:65, :],
                                    cacc[:, 1, :])
                        if w == KT - 1 and norm_cb is not None:
                            norm_cb(cacc)
                    return go

                for kt in range(KT):
                    cur_g[0] = li * KT + kt
                    psc = psum.tile([128, 1024], dt32, tag=sc_tags[kt % 2],
                                    name="psc")
                    nc.tensor.matmul(
                        psc[:, 0:512],
                        kts[0:64, hp, kt * 128:(kt + 1) * 128],
                        qts[0:64, hp, q0:q0 + 512],
                        start=True, stop=True, skip_group_check=True,
                    )
                    nc.tensor.matmul(
                        psc[:, 512:1024],
                        kts[64:128, hp, kt * 128:(kt + 1) * 128],
                        qts[64:128, hp, q0:q0 + 512],
                        start=True, stop=True, skip_group_check=True,
                    )
                    at = attn.tile([128, 1024], dtb, tag="at", name="at",
                                   bufs=14)
                    nc.scalar.activation(
                        out=at[:],
                        in_=psc[:],
                        func=mybir.ActivationFunctionType.Exp,
                        scale=0.125,
                    )
                    pvq.append((hp, kt // 4, mk_pv(kt, at)))
                    drain_pv(2)
                    drain_fill(budget)
                return cacc

            # ---- preamble projections (inline; borrow banks free
            # until the fillers need them) ----
            boxk, boxq = {}, {}
            proj_qk_half(0, xkc[0], kts, 0, 0, boxk, 0)()
            proj_qk_half(0, xkc[0], kts, 0, 0, boxk, 1)()
            proj_qk_half(1, xqc[0], qts, 0, 0, boxq, 0)()
            proj_qk_half(1, xqc[0], qts, 0, 0, boxq, 1)()

            # ---- filler schedule ----
            # scores(kt) of loop li read kts chunk kt//4 / qts chunk qc: the
            # writing filler must drain strictly before that score's
            # emission slot.
            push_proj_qk(0, xkc, kts, 1, 0, 2, 3)      # by kt4 of L0
            push_proj_qk(0, xkc, kts, 2, 0, 5, 7)      # by kt8
            push_proj_qk(0, xkc, kts, 3, 0, 9, 11)     # by kt12
            # K ot1 (needed by L4, g64)
            push_proj_qk(0, xkc, kts, 0, 1, 15, 16)
            push_proj_qk(0, xkc, kts, 1, 1, 17, 18)
            push_proj_qk(0, xkc, kts, 2, 1, 19, 20)
            push_proj_qk(0, xkc, kts, 3, 1, 21, 23)
            # V hp0 (gates PV of loops 0-3 via v_ready)
            push_proj_v(0, 0, 8, 9)
            push_proj_v(1, 0, 12, 13)
            push_proj_v(2, 0, 24, 25)
            push_proj_v(3, 0, 26, 27)
            # Q ot0 (gates scores of loops 1-3; must drain before loop start)
            push_proj_qk(1, xqc, qts, 1, 0, 10, 11)    # L1 starts g16
            push_proj_qk(1, xqc, qts, 2, 0, 28, 29)    # L2 starts g32
            push_proj_qk(1, xqc, qts, 3, 0, 44, 45)    # L3 starts g48
            # Q ot1 (gates scores of loops 4-7)
            push_proj_qk(1, xqc, qts, 0, 1, 37, 38)    # L4 starts g64
            push_proj_qk(1, xqc, qts, 1, 1, 40, 41)
            push_proj_qk(1, xqc, qts, 2, 1, 58, 59)
            push_proj_qk(1, xqc, qts, 3, 1, 60, 61)
            # V hp1 (gates PV of loops 4-7 via v_ready)
            push_proj_v(0, 1, 53, 54)
            push_proj_v(1, 1, 55, 56)
            push_proj_v(2, 1, 57, 58)
            push_proj_v(3, 1, 59, 60)

            # ---- run the 8 loops (head-pair-major) ----
            loops = [(0, 0), (0, 1), (0, 2), (0, 3),
                     (1, 0), (1, 1), (1, 2), (1, 3)]

            for li, (hp, qc) in enumerate(loops):
                last = li == len(loops) - 1
                if last:
                    cb = None
                else:
                    def cb(cacc, li=li, qc=qc, hp=hp):
                        norm_start_finish(li, qc, hp, cacc)
                cacc_last = attn_loop(li, qc, hp, cb)

            # ---- tail ----
            # Drain the remaining PVs FIRST (all v_ready by now), then emit
            # the tail norm while its accumulators are still unclobbered;
            # only then drain leftover fillers (whose borrowed banks rotate
            # over all four pv tags).
            cur_g[0] = 10 ** 9
            guard = 0
            while pvq and guard < 1000:
                if drain_pv(len(pvq) + 1) == 0:
                    drain_fill(len(fillers) + 1)
                guard += 1
            assert not pvq, "undrained PV matmuls at tail"
            do_norm_tail(3, 1, cacc_last)
            drain_everything()
            push_wo(3, 0, tail=True)
            drain_everything()

    _split_waits(nc)
    return nc


def _get_program():
    global _PROGRAM
    if _PROGRAM is None:
        _PROGRAM = _build_program()
    return _PROGRAM


# ---------------------------------------------------------------- host side
def kernel(**inputs):
    global LAST_EXEC_NS
    queries = np.asarray(inputs["queries"], np.float32)
    keys = np.asarray(inputs["keys"], np.float32)
    values = np.asarray(inputs["values"], np.float32)
    Wq = np.asarray(inputs["Wq"], np.float32)
    Wk = np.asarray(inputs["Wk"], np.float32)
    Wv = np.asarray(inputs["Wv"], np.float32)
    Wo = np.asarray(inputs["Wo"], np.float32)

    def tile_x(xb):
        # [D, S] -> [c, p, pc, ko, s]: one fully-contiguous 0.5MB DMA per
        # s-chunk
        t = xb.T.astype(np.float16).reshape(2, 4, 128, 4, 512)
        return np.ascontiguousarray(t.transpose(3, 2, 0, 1, 4))

    def tile_w(W, rows):
        # W[rows].T [D, DL] -> [ot, p, kc, o]
        t = W[rows, :].T.astype(np.float16).reshape(8, 128, 2, 128)
        return np.ascontiguousarray(t.transpose(2, 1, 0, 3))

    xqs = [tile_x(queries[b]) for b in range(B)]
    xks = [tile_x(keys[b]) for b in range(B)]
    xvs = [tile_x(values[b]) for b in range(B)]

    in_maps = []
    for c in range(N_CORES):
        b, g = c // 4, c % 4
        rows = slice(g * DL, (g + 1) * DL)
        woT = Wo[:, rows].T.reshape(HL, HD, D)
        wo_p = np.ascontiguousarray(
            np.stack(
                [np.concatenate([woT[2 * p], woT[2 * p + 1]], axis=0) for p in range(2)],
                axis=0,
            ).transpose(1, 0, 2).astype(np.float16)
        )
        in_maps.append({
            "xq": xqs[b],
            "xk": xks[b],
            "xv": xvs[b],
            # [ot, p, which(0=K,1=Q), kc, o]
            "wqk": np.ascontiguousarray(np.stack(
                [tile_w(Wk, rows), tile_w(Wq, rows)], axis=2)),
            "wv": tile_w(Wv, rows),  # [hp, p, kc, o]: dl = hp*128+o matches
            "wo": wo_p,
        })

    nc = _get_program()
    res = bass_utils.run_bass_kernel_spmd(
        nc, in_maps, list(range(N_CORES)), trace=TRACE
    )
    if TRACE:
        LAST_EXEC_NS = res.exec_time_ns

    full = np.zeros((B, S, D), np.float32)
    for b in range(B):
        acc = res.results[b * 4 + 0]["out"].astype(np.float32)
        for g in range(1, 4):
            acc = acc + res.results[b * 4 + g]["out"].astype(np.float32)
        # [qc, t, jc, p, s] -> [S, D]
        full[b] = acc.transpose(0, 1, 3, 2, 4).reshape(S, D)
    return full


# revision 46
# speedup vs baseline: 1.0373x; 1.0373x over previous
"""Multi-head attention kernel for 8 Trainium2 NeuronCores (v3).

Problem: B=2, S=2048, D=1024, H=16 heads (head_dim 64).
Sharding: data-parallel over batch (2) x tensor-parallel over heads (4 groups
of 4 heads). Core c handles batch c//4, heads [4*(c%4), 4*(c%4)+4).
Each core computes a partial [S, D] output (its heads' contribution through
Wo); the host sums the 4 TP partials per batch.

v3 redesign vs v2 (v2 ran projections serially before attention, leaving the
ACT engine idle for the first ~50us of a ~257us kernel):
- The exp stream IS the kernel: 128 ACT exps of [128,1024] score tiles
  (~142us serial ACT work). Everything else hides under it.
- Loops run head-pair-major ((h0,q0..q3) then (h1,q0..q3)) so only the ot0
  halves of the K/Q projections gate the first 4 loops; ot1 halves, V-hp1
  and all Wo work fill later loops.
- First exp fires ~9us in, gated only by K-chunk0-ot0 + Q-chunk0-ot0
  (DMAs split across the three DGE rings) + an 8-matmul projection each.
- PV matmuls are deferred through a V-readiness-gated FIFO: exps never wait
  for V. `at` probability tiles are retained (deep ring) until PV drains.
- Projections/Wo run as slot-gated filler closures in the PE stream, sized
  ~4 matmuls, borrowing the idle PV PSUM bank set. Fillers a loop's scores
  depend on are scheduled strictly before that loop's emission point
  (program order = dependency order in Tile).
- Softmax 1/den off the in-order ACT queue: DVE reciprocal on the PSUM den
  row, DRAM-roundtrip partition broadcast, DVE normalize-multiplies; the
  norm emission chains off the loop's 16th PV matmul. Only the tail norm
  stays on ACT (Ln/Exp + PE rank-1 broadcast) for latency.
- fp16 everywhere (fp32 PSUM). fp8/DoubleRow was measured (micro.py) but
  rejected: e4m3 quantization on the incoherently-summed PV/Wo paths adds
  ~2-5% relative sigma, over the 2e-2 absmax gate.
"""
import sys

sys.path.insert(0, "/opt/trn_rl_repo")

from collections import deque

import numpy as np

import concourse.bass as bass
import concourse.tile as tile
from concourse import mybir
from concourse import bass_utils

# no fish share in this container; only used when tracing
bass_utils.upload_artifacts = lambda tmpdir: f"local://{tmpdir}"

B, S, D, H = 2, 2048, 1024, 16
HD = 64          # head dim
HL = 4           # heads per core (local)
DL = HL * HD     # local projection dim = 256
N_CORES = 8
SC = 4           # s-chunks of 512 for projections
QC = 4           # q-chunks of 512 for attention
KT = 16          # k-tiles of 128

dt32 = mybir.dt.float32
dtb = mybir.dt.float16

TRACE = False           # set by test.py for profiling runs
LAST_EXEC_NS = None     # stashed by kernel() when TRACE


# ---------------------------------------------------------------- wait split
def _split_waits(nc):
    """Walrus codegen accepts at most one sync wait per instruction on this
    toolchain; move excess waits onto same-engine NoOps inserted before the
    overloaded instruction (engine program order makes this equivalent)."""
    n = 0
    for bb_wrap in nc.main_func.blocks:
        bb = bb_wrap if not hasattr(bb_wrap, "bb") else bb_wrap.bb
        insts = list(bb.instructions)
        out = []
        for ins in insts:
            si = ins.sync_info
            waits = list(si.on_wait) if si is not None else []
            if len(waits) > 1:
                for w in waits[:-1]:
                    nop = mybir.InstNoOp(
                        name=nc.get_next_instruction_name(), ins=[], outs=[]
                    )
                    nop.engine = ins.engine
                    nop.sync_info = mybir.SyncInfo(on_wait=[w], on_update=[])
                    nc.register_instruction(nop)
                    out.append(nop)
                    n += 1
                ins.sync_info = mybir.SyncInfo(
                    on_wait=waits[-1:], on_update=list(si.on_update)
                )
            out.append(ins)
        if len(out) != len(insts):
            bb.instructions = out
    return n


# ---------------------------------------------------------------- program
_PROGRAM = None


def _build_program():
    nc = bass.Bass()
    # All inputs/outputs are pre-tiled on the host so every DMA is a
    # fully contiguous block: strided DMAs cost 2-7us of SEQUENCER time to
    # issue (per-row descriptors) and transfer far below HBM rate.
    xq = nc.declare_dram_parameter("xq", [SC, 128, 2, 4, 512], dtb, isOutput=False)
    xk = nc.declare_dram_parameter("xk", [SC, 128, 2, 4, 512], dtb, isOutput=False)
    xv = nc.declare_dram_parameter("xv", [SC, 128, 2, 4, 512], dtb, isOutput=False)
    # wq+wk packed ot-major: one 0.5MB DMA per ot half
    wqk = nc.declare_dram_parameter("wqk", [2, 128, 2, 8, 128], dtb, isOutput=False)
    wv = nc.declare_dram_parameter("wv", [2, 128, 8, 128], dtb, isOutput=False)
    wo = nc.declare_dram_parameter("wo", [128, 2, D], dtb, isOutput=False)
    out = nc.declare_dram_parameter("out", [QC, 4, 2, 128, 512], dtb, isOutput=True)

    with tile.TileContext(nc) as tc:
        with tc.tile_pool(name="const", bufs=1) as const, \
             tc.tile_pool(name="persist", bufs=1) as persist, \
             tc.tile_pool(name="xin", bufs=1) as xin, \
             tc.tile_pool(name="attn", bufs=1) as attn, \
             tc.tile_pool(name="recp", bufs=2) as recp, \
             tc.tile_pool(name="acc", bufs=3) as acc, \
             tc.tile_pool(name="outsb", bufs=4) as outsb, \
             tc.tile_pool(name="dram", bufs=1, space="DRAM") as dram, \
             tc.tile_pool(name="psum", bufs=1, space="PSUM") as psum:

            # ---- resident weights & activations ----
            # [p, ot, which(0=K,1=Q), kc, o]
            wqks = const.tile([128, 2, 2, 8, 128], dtb, tag="wqk")
            wvs = const.tile([128, 2, 8, 128], dtb, tag="wv")  # [p, hp, kc, o]
            wos = const.tile([128, 2, D], dtb, tag="wo")
            qts = persist.tile([128, 2, S], dtb, tag="qts")  # [o%128, o//128, s]
            kts = persist.tile([128, 2, S], dtb, tag="kts")
            # V with a trailing ones column: stationary [v | 1] gives the
            # softmax denominator as PSUM row 64 of the PV accumulation.
            vtsE = persist.tile([128, KT, 2, 66], dtb, tag="vtsE")  # even heads
            vtsO = persist.tile([128, KT, 2, 66], dtb, tag="vtsO")  # odd heads
            # ctx2: head-pair ctx stacked across partitions (even head rows
            # 0-63 from the DVE normalize; odd head rows 64-127 filled by an
            # SBUF->SBUF partition-shift DMA from ctxO) so Wo runs as
            # 2x K=128 matmuls per out tile instead of 4x K=64.
            ctx2 = persist.tile([128, 8, 512], dtb, tag="ctx2")  # slot = qc*2+hp
            ctxO = persist.tile([HD, 2, 512], dtb, tag="ctxO")  # slot%2 ring

            # ---- input DMAs, split across the three DGE rings ----
            def x_chunk(xdram, c, tag, queue):
                t = xin.tile([128, 2, 4, 512], dtb, tag=tag, name=tag, bufs=1)
                queue.dma_start(out=t[:], in_=xdram[c])
                return t

            xkc, xqc, xvc = {}, {}, {}

            def x_chunk_split(xdram, c, tag, q0, q1):
                # two ring-parallel 0.25MB halves into one chunk tile
                t = xin.tile([128, 2, 4, 512], dtb, tag=tag, name=tag, bufs=1)
                q0.dma_start(out=t[:, 0], in_=xdram[c, :, 0])
                q1.dma_start(out=t[:, 1], in_=xdram[c, :, 1])
                return t

            # sync ring (first: the startup-critical halves)
            xkc[0] = x_chunk_split(xk, 0, "xk0", nc.sync, nc.gpsimd)
            xqc[0] = x_chunk_split(xq, 0, "xq0", nc.sync, nc.gpsimd)
            xkc[2] = x_chunk(xk, 2, "xk2", nc.sync)
            xvc[0] = x_chunk(xv, 0, "xv0", nc.sync)
            xvc[2] = x_chunk(xv, 2, "xv2", nc.sync)
            xqc[2] = x_chunk(xq, 2, "xq2", nc.sync)
            nc.sync.dma_start(out=wos[:], in_=wo[:])
            # scalar (ACT) ring: weights ONLY, all issued before the exp
            # stream starts -- a blocked DMA issue on this ring stalls the
            # ACT sequencer and with it every exp.
            nc.scalar.dma_start(out=wqks[:, 0], in_=wqk[0])
            nc.scalar.dma_start(out=wqks[:, 1], in_=wqk[1])
            nc.scalar.dma_start(out=wvs[:, 0], in_=wv[0])
            nc.scalar.dma_start(out=wvs[:, 1], in_=wv[1])
            # gpsimd ring (xk0/xq0 second halves were enqueued above)
            xkc[1] = x_chunk(xk, 1, "xk1", nc.gpsimd)
            xqc[1] = x_chunk(xq, 1, "xq1", nc.gpsimd)
            xkc[3] = x_chunk(xk, 3, "xk3", nc.gpsimd)
            xvc[1] = x_chunk(xv, 1, "xv1", nc.gpsimd)
            xvc[3] = x_chunk(xv, 3, "xv3", nc.gpsimd)
            xqc[3] = x_chunk(xq, 3, "xq3", nc.gpsimd)

            # ---- constants ----
            nc.vector.memset(vtsE[:], 1.0)
            nc.vector.memset(vtsO[:], 1.0)
            # bias const for the tail's exp-based reciprocal: -12*ln(2)
            nbias = const.tile([128, 1], dt32, tag="nbias")
            nc.vector.memset(nbias[:], -12.0 * 0.6931471805599453)
            # f16 ones for the tail's PE-broadcast of 1/den
            ones16 = const.tile([128, HD], dtb, tag="ones16")
            nc.vector.memset(ones16[:], 1.0)
            # dummy exp+ln to pull the act-table load into the preamble
            warm = const.tile([128, 2], dt32, tag="warm")
            nc.vector.memset(warm[:], 1.0)
            nc.scalar.activation(out=warm[0:1, 0:1], in_=warm[0:1, 0:1],
                                 func=mybir.ActivationFunctionType.Exp)
            nc.scalar.activation(out=warm[0:1, 1:2], in_=warm[0:1, 1:2],
                                 func=mybir.ActivationFunctionType.Ln)

            # ---- filler machinery ----
            # g = global slot = loop*16 + kt. fillers: slot-gated closures
            # run in the PE stream. pvq: PV matmul closures gated on their V
            # chunk's projection having RUN (v_ready), FIFO order.
            fillers = []          # list of [min_g, ready_fn, fn]
            pvq = deque()         # (hp, chunk, fn)
            v_ready = {(hp, c): False for hp in range(2) for c in range(SC)}
            cur_g = [0]

            def drain_pv(budget):
                done = 0
                while pvq and done < budget:
                    hp, c, fn = pvq[0]
                    if not v_ready[(hp, c)]:
                        break
                    pvq.popleft()
                    fn()
                    done += 1
                return done

            def drain_fill(budget):
                # Pick the eligible filler with the SMALLEST min_g: norm-fin
                # closures must preempt bank-borrowing fillers pushed earlier.
                done = 0
                while done < budget:
                    best, best_i = None, -1
                    for i, (mg, rdy, fn) in enumerate(fillers):
                        if mg > cur_g[0]:
                            continue
                        if rdy is not None and not rdy():
                            continue
                        if best is None or mg < best:
                            best, best_i = mg, i
                    if best_i < 0:
                        break
                    mg, rdy, fn = fillers.pop(best_i)
                    fn()
                    done += 1
                return done

            def push(min_g, fn, ready=None):
                fillers.append([min_g, ready, fn])

            def drain_everything():
                cur_g[0] = 10 ** 9
                guard = 0
                while (pvq or fillers) and guard < 10000:
                    drain_pv(len(pvq) + 1)
                    drain_fill(len(fillers) + 1)
                    guard += 1

            # PSUM: sc0/sc1 are [128,1024] score double-buffers (4 banks).
            # pvA/pvB are THE PV accumulator pair for every loop: PV runs in
            # groups of 8 kt, each group evict/added into an SBUF fp16
            # accumulator (DVE), so the banks free deterministically inline.
            # pvA2/pvB2 are a dedicated filler pair, always borrowable.
            fill_rot = [0]

            def next_fill_tag():
                if cur_g[0] >= 8 * KT:  # tail: rotate all four
                    tags = ("pvA", "pvB", "pvA2", "pvB2")
                    t = tags[fill_rot[0] % 4]
                else:
                    t = ("pvA2", "pvB2")[fill_rot[0] % 2]
                fill_rot[0] += 1
                return t

            # ---- projection builders ----
            def proj_qk_half(wsel, xct, dst, c, ot, box, half):
                """wsel: 0=K, 1=Q. half 0: kc0-3 into a fresh borrowed
                bank; half 1: kc4-7 + cast eviction."""
                def go():
                    if "p" not in box:
                        box["p"] = psum.tile([128, 512], dt32,
                                             tag=next_fill_tag(),
                                             name=f"pj{c}o{ot}")
                    p = box["p"]
                    kcs = range(0, 4) if half == 0 else range(4, 8)
                    for kc in kcs:
                        nc.tensor.matmul(
                            p[:],
                            wqks[:, ot, wsel, kc, :],
                            xct[:, kc // 4, kc % 4, :],
                            start=(kc == 0),
                            stop=(kc == 7),
                            skip_group_check=True,
                        )
                    if half == 1:
                        nc.vector.tensor_copy(
                            dst[:, ot, c * 512:(c + 1) * 512], p[:])
                return go

            def push_proj_qk(wsel, chunk_map, dst, c, ot, g0, g1):
                box = {}

                def half(h):
                    def go():
                        proj_qk_half(wsel, chunk_map[c], dst, c, ot, box, h)()
                    return go

                push(g0, half(0))
                push(g1, half(1))

            def proj_v_half(c, hp, box, half):
                """V proj produces [s-sub, head dl] (s on partitions). The hp
                half moves wvs cols [hp*128:(hp+1)*128] (heads 2hp, 2hp+1).
                half 0: s-subtiles 0,1; half 1: subtiles 2,3 + v_ready."""
                xct = xvc[c]

                def go():
                    if "p" not in box:
                        box["p"] = psum.tile([128, 512], dt32,
                                             tag=next_fill_tag(),
                                             name=f"pv{c}h{hp}")
                    p = box["p"]
                    sts = (0, 1) if half == 0 else (2, 3)
                    for st in sts:
                        for kc in range(8):
                            nc.tensor.matmul(
                                p[:, st * 128:(st + 1) * 128],
                                xct[:, kc // 4, kc % 4, st * 128:(st + 1) * 128],
                                wvs[:, hp, kc, :],
                                start=(kc == 0),
                                stop=(kc == 7),
                                skip_group_check=True,
                            )
                        idx = c * 4 + st
                        ph = p[:, st * 128:(st + 1) * 128].rearrange(
                            "p (h d) -> p h d", h=2)
                        nc.vector.tensor_copy(vtsE[:, idx, hp, 0:HD], ph[:, 0, :])
                        nc.vector.tensor_copy(vtsO[:, idx, hp, 0:HD], ph[:, 1, :])
                    if half == 1:
                        v_ready[(hp, c)] = True
                return go

            def push_proj_v(c, hp, g0, g1):
                box = {}
                push(g0, proj_v_half(c, hp, box, 0))
                push(g1, proj_v_half(c, hp, box, 1))

            # ---- softmax normalization (off-ACT path) ----
            def norm_start_finish(li, qc, hp, cacc):
                """Emit DVE reciprocal + DRAM-roundtrip broadcast now (called
                right after the loop's last PV group lands in the SBUF
                accumulator); schedule the normalize-multiplies + partition
                shift + (for hp1) Wo as fillers a few slots later."""
                slot = qc * 2 + hp
                rec32 = recp.tile([128, 2, 512], dt32, tag="rec32",
                                  name=f"rec{slot}")
                # The DVE reciprocal is ~6.4ns/elem; emitting it whole
                # blocks the DVE for 6.6us and stalls PE fillers WAR-ing on
                # DVE evictions. Split into 2 chunks drained between
                # fillers; the broadcast roundtrip goes fp16.
                g = cur_g[0]
                for hh in range(2):

                    def rc(hh=hh):
                        nc.vector.reciprocal(
                            rec32[64:65, hh, :], cacc[64:65, hh, :])
                    push(g + 1 + 2 * hh, rc)

                rec16 = recp.tile([128, 2, 512], dtb, tag="rec16",
                                  name=f"rec16_{slot}")
                recd = dram.tile([2, 512], dtb, tag=f"recd{slot}",
                                 name=f"recd{slot}")
                denbS = recp.tile([HD, 2, 512], dtb, tag="denbS",
                                  name=f"denbS{slot}")

                def bcast():
                    nc.vector.tensor_copy(rec16[64:65, :, :],
                                          rec32[64:65, :, :])
                    nc.sync.dma_start(out=recd[:], in_=rec16[64:65, :, :])
                    for hh in range(2):
                        row = recd[hh:hh + 1, :]
                        bc = bass.AP(
                            tensor=row.tensor,
                            offset=row.offset,
                            ap=[[0, HD]] + [list(x) for x in row.ap[1:]],
                        )
                        nc.sync.dma_start(out=denbS[:, hh, :], in_=bc)
                push(g + 4, bcast)

                def fin():
                    nc.gpsimd.tensor_mul(
                        ctx2[0:HD, slot, :], cacc[0:HD, 0, :], denbS[:, 0, :])
                    nc.gpsimd.tensor_mul(
                        ctxO[:, slot % 2, :], cacc[0:HD, 1, :], denbS[:, 1, :])
                    nc.sync.dma_start(
                        out=ctx2[HD:128, slot, :], in_=ctxO[:, slot % 2, :])
                    if hp == 1:
                        # the fin data chain (roundtrip DMA + mults + shift)
                        # takes ~7us; an early Wo matmul head-blocks the PE
                        push_wo(qc, cur_g[0] + 8)
                push(g + 6, fin)

            def do_norm_tail(qc, hp, cacc):
                # tail-only: ACT is idle after the last exp and PE is nearly
                # free; Ln/Exp reciprocal + rank-1 PE broadcast beats the
                # DVE-recip + DRAM-roundtrip latency chain.
                slot = qc * 2 + hp
                lnden = recp.tile([128, 2, 512], dt32, tag="rec32", name="lndent")
                rec16 = recp.tile([128, 2, 512], dtb, tag="rec16t", name="rec16t", bufs=1)
                nc.scalar.activation(
                    out=lnden[64:65, 0, :], in_=cacc[64:65, 0, :],
                    func=mybir.ActivationFunctionType.Ln, scale=2.0 ** -12,
                )
                nc.scalar.activation(
                    out=lnden[64:65, 1, :], in_=cacc[64:65, 1, :],
                    func=mybir.ActivationFunctionType.Ln, scale=2.0 ** -12,
                )
                nc.scalar.activation(
                    out=rec16[64:65, :, :], in_=lnden[64:65, :, :],
                    func=mybir.ActivationFunctionType.Exp, scale=-1.0,
                    bias=nbias[64:65, :],
                )
                denbE = psum.tile([128, 512], dt32, tag="pvA", name="denbE")
                denbO = psum.tile([128, 512], dt32, tag="pvB", name="denbO")
                nc.tensor.matmul(
                    denbE[0:HD, :], ones16[64:65, :], rec16[64:65, 0, :],
                    start=True, stop=True, skip_group_check=True,
                )
                nc.tensor.matmul(
                    denbO[0:HD, :], ones16[64:65, :], rec16[64:65, 1, :],
                    start=True, stop=True, skip_group_check=True,
                )
                denbS = recp.tile([HD, 2, 512], dt32, tag="denbSt", name="denbSt", bufs=1)
                nc.vector.tensor_copy(denbS[:, 0, :], denbE[0:HD, :])
                nc.vector.tensor_copy(denbS[:, 1, :], denbO[0:HD, :])
                nc.vector.tensor_mul(
                    ctx2[0:HD, slot, :], cacc[0:HD, 0, :], denbS[:, 0, :])
                nc.vector.tensor_mul(
                    ctxO[:, slot % 2, :], cacc[0:HD, 1, :], denbS[:, 1, :])
                # split the partition-shift over two rings for tail latency
                nc.sync.dma_start(
                    out=ctx2[HD:HD + 32, slot, :], in_=ctxO[0:32, slot % 2, :])
                nc.gpsimd.dma_start(
                    out=ctx2[HD + 32:128, slot, :], in_=ctxO[32:64, slot % 2, :])

            # ---- Wo fillers ----
            def push_wo(qc, g0, tail=False):
                for idx in range(8):
                    t, jc = idx // 2, idx % 2

                    def mk(t, jc, idx):
                        def go():
                            po = psum.tile(
                                [128, 512], dt32,
                                tag=next_fill_tag(), name=f"po{qc}")
                            for p in range(2):
                                nc.tensor.matmul(
                                    po[:],
                                    ctx2[:, qc * 2 + p, t * 128:(t + 1) * 128],
                                    wos[:, p, jc * 512:(jc + 1) * 512],
                                    start=(p == 0),
                                    stop=(p == 1),
                                    skip_group_check=True,
                                )
                            ob = outsb.tile([128, 512], dtb, tag="ob", name="ob")
                            if tail:
                                # ACT engine is idle after the last exp; use
                                # it for eviction so the tail isn't
                                # DVE-serialized, and split out-DMAs over
                                # two rings
                                nc.scalar.copy(ob[:], po[:])
                            else:
                                nc.vector.tensor_copy(ob[:], po[:])
                            (nc.sync if (tail and idx % 2) else nc.gpsimd).dma_start(
                                out=out[qc, t, jc], in_=ob[:],
                            )
                        return go

                    push(g0 + (3 * idx) // 2, mk(t, jc, idx))

            # ---- attention loop ----
            sc_tags = ("sc0", "sc1")

            def attn_loop(li, qc, hp, norm_cb):
                # sweep fillers scheduled before this loop's start so the
                # scores below are emitted after (= depend on) their
                # projections
                cur_g[0] = li * KT
                drain_fill(len(fillers) + 1)
                budget = 2 if li <= 1 or li == 7 else 1

                # PV accumulates in pvA/pvB in two groups of 8 kt; each
                # group is evicted (group 0: copy, group 1: in-place add)
                # into the SBUF fp16 accumulator, freeing the banks inline.
                cacc = acc.tile([65, 2, 512], dtb, tag="cacc",
                                name=f"cacc{li}")
                q0 = qc * 512
                nmm = [0]
                pvt = {}

                def mk_pv(kt, at):
                    def go():
                        w = nmm[0]
                        nmm[0] += 1
                        if w % 8 == 0:
                            pvt["E"] = psum.tile([128, 512], dt32, tag="pvA",
                                                 name=f"pvE{li}g{w // 8}")
                            pvt["O"] = psum.tile([128, 512], dt32, tag="pvB",
                                                 name=f"pvO{li}g{w // 8}")
                        first, last = w % 8 == 0, w % 8 == 7
                        nc.tensor.matmul(
                            pvt["E"][0:65, :], vtsE[:, kt, hp, 0:65],
                            at[:, 0:512],
                            start=first, stop=last, skip_group_check=True,
                        )
                        nc.tensor.matmul(
                            pvt["O"][0:65, :], vtsO[:, kt, hp, 0:65],
                            at[:, 512:1024],
                            start=first, stop=last, skip_group_check=True,
                        )
                        if w % 8 == 7:
                            if w // 8 == 0:
                                nc.vector.tensor_copy(
                                    cacc[:, 0, :], pvt["E"][0:65, :])
                                nc.vector.tensor_copy(
                                    cacc[:, 1, :], pvt["O"][0:65, :])
                            else:
                                nc.vector.tensor_add(
                                    cacc[:, 0, :], pvt["E"][0:65, :],
                                    cacc[:, 0, :])
                                nc.vector.tensor_add(
                                    cacc[:, 1, :], pvt["O"][0:65, :],
                                    cacc[:, 1, :])
                        if w == KT - 1 and norm_cb is not None:
                            norm_cb(cacc)
                    return go

                for kt in range(KT):
                    cur_g[0] = li * KT + kt
                    psc = psum.tile([128, 1024], dt32, tag=sc_tags[kt % 2],
                                    name="psc")
                    nc.tensor.matmul(
                        psc[:, 0:512],
                        kts[0:64, hp, kt * 128:(kt + 1) * 128],
                        qts[0:64, hp, q0:q0 + 512],
                        start=True, stop=True, skip_group_check=True,
                    )
                    nc.tensor.matmul(
                        psc[:, 512:1024],
                        kts[64:128, hp, kt * 128:(kt + 1) * 128],
                        qts[64:128, hp, q0:q0 + 512],
                        start=True, stop=True, skip_group_check=True,
                    )
                    at = attn.tile([128, 1024], dtb, tag="at", name="at",
                                   bufs=14)
                    nc.scalar.activation(
                        out=at[:],
                        in_=psc[:],
                        func=mybir.ActivationFunctionType.Exp,
                        scale=0.125,
                    )
                    pvq.append((hp, kt // 4, mk_pv(kt, at)))
                    drain_pv(2)
                    drain_fill(budget)
                return cacc

            # ---- preamble projections (inline; borrow banks free
            # until the fillers need them) ----
            boxk, boxq = {}, {}
            proj_qk_half(0, xkc[0], kts, 0, 0, boxk, 0)()
            proj_qk_half(0, xkc[0], kts, 0, 0, boxk, 1)()
            proj_qk_half(1, xqc[0], qts, 0, 0, boxq, 0)()
            proj_qk_half(1, xqc[0], qts, 0, 0, boxq, 1)()

            # ---- filler schedule ----
            # scores(kt) of loop li read kts chunk kt//4 / qts chunk qc: the
            # writing filler must drain strictly before that score's
            # emission slot.
            push_proj_qk(0, xkc, kts, 1, 0, 2, 3)      # by kt4 of L0
            push_proj_qk(0, xkc, kts, 2, 0, 5, 7)      # by kt8
            push_proj_qk(0, xkc, kts, 3, 0, 9, 11)     # by kt12
            # K ot1 (needed by L4, g64)
            push_proj_qk(0, xkc, kts, 0, 1, 33, 34)
            push_proj_qk(0, xkc, kts, 1, 1, 35, 36)
            push_proj_qk(0, xkc, kts, 2, 1, 37, 38)
            push_proj_qk(0, xkc, kts, 3, 1, 39, 41)
            # V hp0 (gates PV of loops 0-3 via v_ready)
            push_proj_v(0, 0, 8, 9)
            push_proj_v(1, 0, 12, 13)
            push_proj_v(2, 0, 24, 25)
            push_proj_v(3, 0, 26, 27)
            # Q ot0 (gates scores of loops 1-3; must drain before loop start)
            push_proj_qk(1, xqc, qts, 1, 0, 10, 11)    # L1 starts g16
            push_proj_qk(1, xqc, qts, 2, 0, 28, 29)    # L2 starts g32
            push_proj_qk(1, xqc, qts, 3, 0, 44, 45)    # L3 starts g48
            # Q ot1 (gates scores of loops 4-7)
            push_proj_qk(1, xqc, qts, 0, 1, 42, 43)    # L4 starts g64
            push_proj_qk(1, xqc, qts, 1, 1, 46, 47)
            push_proj_qk(1, xqc, qts, 2, 1, 58, 59)
            push_proj_qk(1, xqc, qts, 3, 1, 60, 61)
            # V hp1 (gates PV of loops 4-7 via v_ready)
            push_proj_v(0, 1, 56, 57)
            push_proj_v(1, 1, 62, 63)
            push_proj_v(2, 1, 66, 67)
            push_proj_v(3, 1, 68, 69)

            # ---- run the 8 loops (head-pair-major) ----
            loops = [(0, 0), (0, 1), (0, 2), (0, 3),
                     (1, 0), (1, 1), (1, 2), (1, 3)]

            for li, (hp, qc) in enumerate(loops):
                last = li == len(loops) - 1
                if last:
                    cb = None
                else:
                    def cb(cacc, li=li, qc=qc, hp=hp):
                        norm_start_finish(li, qc, hp, cacc)
                cacc_last = attn_loop(li, qc, hp, cb)

            # ---- tail ----
            # Drain the remaining PVs FIRST (all v_ready by now), then emit
            # the tail norm while its accumulators are still unclobbered;
            # only then drain leftover fillers (whose borrowed banks rotate
            # over all four pv tags).
            cur_g[0] = 10 ** 9
            guard = 0
            while pvq and guard < 1000:
                if drain_pv(len(pvq) + 1) == 0:
                    drain_fill(len(fillers) + 1)
                guard += 1
            assert not pvq, "undrained PV matmuls at tail"
            do_norm_tail(3, 1, cacc_last)
            drain_everything()
            push_wo(3, 0, tail=True)
            drain_everything()

    _split_waits(nc)
    return nc


def _get_program():
    global _PROGRAM
    if _PROGRAM is None:
        _PROGRAM = _build_program()
    return _PROGRAM


# ---------------------------------------------------------------- host side
def kernel(**inputs):
    global LAST_EXEC_NS
    queries = np.asarray(inputs["queries"], np.float32)
    keys = np.asarray(inputs["keys"], np.float32)
    values = np.asarray(inputs["values"], np.float32)
    Wq = np.asarray(inputs["Wq"], np.float32)
    Wk = np.asarray(inputs["Wk"], np.float32)
    Wv = np.asarray(inputs["Wv"], np.float32)
    Wo = np.asarray(inputs["Wo"], np.float32)

    def tile_x(xb):
        # [D, S] -> [c, p, pc, ko, s]: one fully-contiguous 0.5MB DMA per
        # s-chunk
        t = xb.T.astype(np.float16).reshape(2, 4, 128, 4, 512)
        return np.ascontiguousarray(t.transpose(3, 2, 0, 1, 4))

    def tile_w(W, rows):
        # W[rows].T [D, DL] -> [ot, p, kc, o]
        t = W[rows, :].T.astype(np.float16).reshape(8, 128, 2, 128)
        return np.ascontiguousarray(t.transpose(2, 1, 0, 3))

    xqs = [tile_x(queries[b]) for b in range(B)]
    xks = [tile_x(keys[b]) for b in range(B)]
    xvs = [tile_x(values[b]) for b in range(B)]

    in_maps = []
    for c in range(N_CORES):
        b, g = c // 4, c % 4
        rows = slice(g * DL, (g + 1) * DL)
        woT = Wo[:, rows].T.reshape(HL, HD, D)
        wo_p = np.ascontiguousarray(
            np.stack(
                [np.concatenate([woT[2 * p], woT[2 * p + 1]], axis=0) for p in range(2)],
                axis=0,
            ).transpose(1, 0, 2).astype(np.float16)
        )
        in_maps.append({
            "xq": xqs[b],
            "xk": xks[b],
            "xv": xvs[b],
            # [ot, p, which(0=K,1=Q), kc, o]
            "wqk": np.ascontiguousarray(np.stack(
                [tile_w(Wk, rows), tile_w(Wq, rows)], axis=2)),
            "wv": tile_w(Wv, rows),  # [hp, p, kc, o]: dl = hp*128+o matches
            "wo": wo_p,
        })

    nc = _get_program()
    res = bass_utils.run_bass_kernel_spmd(
        nc, in_maps, list(range(N_CORES)), trace=TRACE
    )
    if TRACE:
        LAST_EXEC_NS = res.exec_time_ns

    full = np.zeros((B, S, D), np.float32)
    for b in range(B):
        acc = res.results[b * 4 + 0]["out"].astype(np.float32)
        for g in range(1, 4):
            acc = acc + res.results[b * 4 + g]["out"].astype(np.float32)
        # [qc, t, jc, p, s] -> [S, D]
        full[b] = acc.transpose(0, 1, 3, 2, 4).reshape(S, D)
    return full
